# revision 3
# baseline (speedup 1.0000x reference)
"""AttentionBlock (GroupNorm -> 1x1 QKV -> softmax attention -> proj -> residual)
on Trainium2, data-parallel over batch: 32 images across 8 NeuronCores (4 per core).

v2: fp8e4m3 DoubleRow matmuls (K=256 per matmul, 0.5 cyc/col) for the
logits and attn@V stages; bf16 for h/Wqk/Wv generation matmuls; bf16 input
(x + proj_b folded on host) and bf16 output; exp with -1.5 bias shift
(cancels in softmax) so fp8 exp outputs stay in e4m3 range.  All PSUM
drains on DVE/ACT (GPSIMD cannot access PSUM on TRN2).

Self-contained: hardcodes B=32, C=256, H=W=32, GROUPS=8, EPS=1e-5.
"""

import numpy as np
import ml_dtypes
import jax
from jax.experimental.shard_map import shard_map
from jax.sharding import Mesh, PartitionSpec

import concourse.bass as bass
import concourse.tile as tile
from concourse import bacc, mybir
from concourse import bass2jax

F32 = mybir.dt.float32
BF16 = mybir.dt.bfloat16
FP8 = mybir.dt.float8e4
AF = mybir.ActivationFunctionType
ALU = mybir.AluOpType
PM = mybir.MatmulPerfMode

NCORES = 8
B = 32
BPC = B // NCORES  # images per core
C = 256
N = 1024           # H*W
G = 8              # groups
GS = C // G        # 32 channels per group
EPS = 1e-5
P = 128
NT = C // P        # 2 channel tiles
SCALE = C ** -0.5  # 1/16
EXPB = -1.5        # exp bias shift; cancels in softmax, keeps e^x < 240

_cached = None


def _build_program(repeat=1):
    nc = bacc.Bacc("TRN2", target_bir_lowering=False, debug=False,
                   num_devices=NCORES)

    # xpb = x + proj_b (folded on host), bf16
    xpb_d = nc.dram_tensor("xpb", [BPC, C, N], BF16, kind="ExternalInput")
    wqk_d = nc.dram_tensor("wqk", [P, NT, 2 * C], BF16, kind="ExternalInput")
    # wv holds (proj_w @ v_w).T -- proj folded into V (attention only mixes
    # spatially: Wp @ (attn @ (Wv h)) = attn @ ((Wp Wv) h))
    wv_d = nc.dram_tensor("wv", [P, NT, C], BF16, kind="ExternalInput")
    sel_d = nc.dram_tensor("sel", [P, NT, G], F32, kind="ExternalInput")
    selb_d = nc.dram_tensor("selb", [P, C], F32, kind="ExternalInput")
    # aff cols: [norm_w (NT), norm_b (NT), proj_b (NT)]
    aff_d = nc.dram_tensor("aff", [P, 3 * NT], F32, kind="ExternalInput")
    ident_d = nc.dram_tensor("ident", [P, P], BF16, kind="ExternalInput")
    out_d = nc.dram_tensor("out", [BPC, C, N], BF16, kind="ExternalOutput")

    with tile.TileContext(nc) as tc:
        with (
            tc.tile_pool(name="consts", bufs=1) as consts,
            tc.tile_pool(name="xp", bufs=4) as xp,
            tc.tile_pool(name="gn", bufs=2) as gn,
            tc.tile_pool(name="hp", bufs=2) as hp,
            tc.tile_pool(name="qkp", bufs=2) as qkp,
            tc.tile_pool(name="vtp", bufs=2) as vtp,
            tc.tile_pool(name="ptp", bufs=2) as ptp,
            tc.tile_pool(name="op", bufs=2) as op,
            tc.tile_pool(name="resp", bufs=2) as resp,
            tc.tile_pool(name="recp", bufs=4) as recp,
            tc.tile_pool(name="psb", bufs=2, space="PSUM") as psb,
            tc.tile_pool(name="psv_p", bufs=1, space="PSUM") as psv_p,
            tc.tile_pool(name="pso_p", bufs=2, space="PSUM") as pso_p,
            tc.tile_pool(name="pst_p", bufs=1, space="PSUM") as pst_p,
        ):
            wqk = consts.tile([P, NT, 2 * C], BF16)
            wv = consts.tile([P, NT, C], BF16)
            sel = consts.tile([P, NT, G], F32)
            selb = consts.tile([P, C], F32)
            aff = consts.tile([P, 3 * NT], F32)
            ident = consts.tile([P, P], BF16)
            expb = consts.tile([P, 1], F32)

            def emit_consts():
                nc.sync.dma_start(sel[:], sel_d.ap())
                nc.sync.dma_start(selb[:], selb_d.ap())
                nc.sync.dma_start(aff[:], aff_d.ap())
                nc.gpsimd.dma_start(wqk[:], wqk_d.ap())
                nc.gpsimd.dma_start(wv[:], wv_d.ap())
                nc.gpsimd.dma_start(ident[:], ident_d.ap())

            def emit_x(img):
                x_sb = xp.tile([P, NT, N], BF16, tag="x")
                xr = xpb_d.ap()[img].rearrange("(t p) n -> p t n", p=P)
                nc.sync.dma_start(x_sb[:], xr[:, :, :])
                return x_sb

            def emit_gn_h(x_sb, first=False):
                """GroupNorm stats on xpb -> per-channel affine -> h (bf16).

                xpb = x + pb, so mean' = mean(x) + pb per channel; the h
                affine needs B = b - (mu_g + pb)*A and the group stats need
                per-channel E[x^2] = var' + (mean' - pb)^2.
                """
                bst = gn.tile([P, NT, 2, 6], F32, tag="bst")
                for t in range(NT):
                    for s in range(2):
                        nc.vector.bn_stats(
                            bst[:, t, s, :], x_sb[:, t, s * 512:(s + 1) * 512])
                cmv = gn.tile([P, NT, 2], F32, tag="cmv")
                for t in range(NT):
                    nc.vector.bn_aggr(cmv[:, t, :], bst[:, t, :, :])
                # ex2 columns: [meanX_c, E[x^2]_c] with meanX = mean' - pb
                ex2 = gn.tile([P, NT, 2], F32, tag="ex2")
                for t in range(NT):
                    nc.vector.tensor_tensor(
                        ex2[:, t, 0:1], cmv[:, t, 0:1],
                        aff[:, 2 * NT + t:2 * NT + t + 1], ALU.subtract)
                    nc.vector.tensor_mul(
                        ex2[:, t, 1:2], ex2[:, t, 0:1], ex2[:, t, 0:1])
                    nc.vector.tensor_add(
                        ex2[:, t, 1:2], ex2[:, t, 1:2], cmv[:, t, 1:2])
                # group stats = (1/GS) * sel.T @ ex2 -> psum [G, 2]
                psg = psv_p.tile([G, 2], F32, tag="g")
                for t in range(NT):
                    nc.tensor.matmul(psg[:], sel[:, t, :], ex2[:, t, :],
                                     start=(t == 0), stop=(t == NT - 1))
                # gsb cols: [mean_g, rstd_g, v, tmp]; rows 8..127 zero (pad
                # for matmul).  rstd via DVE-only Newton rsqrt so Exp stays
                # the single ACT table set.
                gsb = gn.tile([P, 4], F32, tag="gsb")
                nc.vector.memset(gsb[:], 0.0)
                nc.vector.tensor_copy(gsb[0:G, 0:1], psg[:, 0:1])
                nc.vector.tensor_mul(
                    gsb[0:G, 3:4], gsb[0:G, 0:1], gsb[0:G, 0:1])
                nc.vector.tensor_tensor(
                    gsb[0:G, 2:3], psg[:, 1:2], gsb[0:G, 3:4], ALU.subtract)
                nc.vector.tensor_scalar_add(gsb[0:G, 2:3], gsb[0:G, 2:3], EPS)
                nc.vector.reciprocal(gsb[0:G, 3:4], gsb[0:G, 2:3])
                nc.vector.tensor_scalar(
                    gsb[0:G, 1:2], gsb[0:G, 3:4], 1.0, 0.5, ALU.add, ALU.mult)
                for _ in range(2):
                    nc.vector.tensor_mul(
                        gsb[0:G, 3:4], gsb[0:G, 1:2], gsb[0:G, 1:2])
                    nc.vector.tensor_mul(
                        gsb[0:G, 3:4], gsb[0:G, 3:4], gsb[0:G, 2:3])
                    nc.vector.tensor_scalar(
                        gsb[0:G, 3:4], gsb[0:G, 3:4], -0.5, 1.5,
                        ALU.mult, ALU.add)
                    nc.vector.tensor_mul(
                        gsb[0:G, 1:2], gsb[0:G, 1:2], gsb[0:G, 3:4])
                # broadcast group -> channel: selb.T @ gsb -> [c, (mean,rstd)]
                AB = gn.tile([P, NT, 2], F32, tag="AB")
                h_sb = hp.tile([P, NT, N], BF16, tag="h")
                for cu in range(NT):
                    psc = psv_p.tile([P, 2], F32, tag="g")
                    nc.tensor.matmul(psc[:], selb[:, cu * P:(cu + 1) * P],
                                     gsb[:, 0:2], start=True, stop=True)
                    # A = rstd*w ; B = b - (mu_g + pb)*A
                    nc.vector.tensor_mul(
                        AB[:, cu, 0:1], psc[:, 1:2], aff[:, cu:cu + 1])
                    nc.vector.tensor_add(
                        AB[:, cu, 1:2], psc[:, 0:1],
                        aff[:, 2 * NT + cu:2 * NT + cu + 1])
                    nc.vector.tensor_mul(
                        AB[:, cu, 1:2], AB[:, cu, 1:2], AB[:, cu, 0:1])
                    nc.vector.tensor_tensor(
                        AB[:, cu, 1:2], aff[:, NT + cu:NT + cu + 1],
                        AB[:, cu, 1:2], ALU.subtract)
                    # h = A*xpb + B   (bf16, all-SBUF: fast DVE path)
                    nc.vector.tensor_scalar(
                        h_sb[:, cu, :], x_sb[:, cu, :],
                        AB[:, cu, 0:1], AB[:, cu, 1:2], ALU.mult, ALU.add)
                return h_sb

            def _drain(eng, dst, src):
                # psum -> sbuf drain on ACT (scalar.copy) or DVE (tensor_copy)
                if eng is nc.scalar:
                    eng.copy(dst, src)
                else:
                    eng.tensor_copy(dst, src)

            def emit_qkv_a(h_sb, qk8, vto8):
                """k as [P,512] halves via the pso pool (bf16 mms, fp8
                copies split ACT/DVE); vpT via psv pool."""
                def qk_group(ou, engs):
                    for half in range(2):
                        psq = pso_p.tile([P, 512], F32, tag="s")
                        for t in range(NT):
                            nc.tensor.matmul(
                                psq[:],
                                wqk[:, t, ou * P:(ou + 1) * P],
                                h_sb[:, t, half * 512:(half + 1) * 512],
                                start=(t == 0), stop=(t == NT - 1))
                        _drain(engs[half],
                               qk8[:, ou, half * 512:(half + 1) * 512],
                               psq[:])

                def vt_group(j, eng):
                    psv = psv_p.tile([P, 2, C], F32, tag="g")
                    for half in range(2):
                        nk = 2 * j + half
                        for t in range(NT):
                            nc.tensor.matmul(
                                psv[:, half, :],
                                h_sb[:, t, nk * P:(nk + 1) * P],
                                wv[:, t, :],
                                start=(t == 0), stop=(t == NT - 1))
                    _drain(eng, vto8[:, 2 * j:2 * j + 2, 0:C], psv[:])
                    nc.gpsimd.memset(
                        vto8[:, 2 * j:2 * j + 2, C:C + 1], 1.0)

                qk_group(2, (nc.scalar, nc.vector))
                qk_group(3, (nc.scalar, nc.vector))
                vt_group(0, nc.scalar)
                vt_group(1, nc.vector)
                vt_group(2, nc.scalar)
                vt_group(3, nc.vector)

            def emit_qkv_b(h_sb, qk8):
                """q (ou 0,1) as [P,512] halves via the pso pool."""
                for ou in (0, 1):
                    for half, eng in ((0, nc.scalar), (1, nc.vector)):
                        psq = pso_p.tile([P, 512], F32, tag="s")
                        for t in range(NT):
                            nc.tensor.matmul(
                                psq[:],
                                wqk[:, t, ou * P:(ou + 1) * P],
                                h_sb[:, t, half * 512:(half + 1) * 512],
                                start=(t == 0), stop=(t == NT - 1))
                        _drain(eng,
                               qk8[:, ou, half * 512:(half + 1) * 512],
                               psq[:])

            def emit_logits(qk8, closures):
                """logitsT [m, n] = k.T @ q (fp8 DoubleRow);
                PT = exp(logitsT/16 - 1.5) in fp8.  closures[mk] emit the
                neighbouring images' work between mk groups."""
                pt8 = ptp.tile([P, 8, N], FP8, tag="pt")
                for mk in range(8):
                    psl = psb.tile([P, N], F32, tag="b")
                    for s in range(4):
                        nc.tensor.matmul(
                            psl[:, s * 256:(s + 1) * 256],
                            qk8[:, 2:4, mk * P:(mk + 1) * P],
                            qk8[:, 0:2, s * 256:(s + 1) * 256],
                            start=True, stop=True, perf_mode=PM.DoubleRow)
                    nc.scalar.activation(pt8[:, mk, :], psl[:], AF.Exp,
                                         scale=SCALE, bias=expb[:])
                    if closures[mk] is not None:
                        closures[mk]()
                if closures[8] is not None:
                    closures[8]()
                return pt8

            def out_chunk(state, nks):
                """AV DoubleRow matmuls (vto carries a ones column for the
                softmax denominator) + rec + normalize, cu=0 transposes
                lagging one nk."""
                img, vto8, pt8, x_sb, o_sb, pstrow = state
                for nk in nks:
                    pso = pso_p.tile([P, 512], F32, tag="s")
                    for j in range(4):
                        nc.tensor.matmul(
                            pso[:, 0:C + 1],
                            pt8[:, 2 * j:2 * j + 2, nk * P:(nk + 1) * P],
                            vto8[:, 2 * j:2 * j + 2, :],
                            start=(j == 0), stop=(j == 3),
                            perf_mode=PM.DoubleRow)
                    rec = recp.tile([P, 1], F32, tag="rec")
                    nc.vector.reciprocal(rec[:], pso[:, C:C + 1])
                    nc.vector.tensor_scalar_mul(
                        o_sb[:, nk, :], pso[:, 0:C], rec[:])
                    if nk > 0:
                        nc.tensor.transpose(
                            pstrow[:, (nk - 1) * P:nk * P],
                            o_sb[:, nk - 1, 0:P], ident[:])

            def out_tail(state):
                img, vto8, pt8, x_sb, o_sb, pstrow = state
                res_sb = resp.tile([P, NT, N], BF16, tag="res")
                outr = out_d.ap()[img].rearrange("(t p) n -> p t n", p=P)
                nc.tensor.transpose(
                    pstrow[:, 7 * P:8 * P], o_sb[:, 7, 0:P], ident[:])
                nc.vector.tensor_tensor(
                    res_sb[:, 0, :], pstrow[:], x_sb[:, 0, :], ALU.add)
                nc.sync.dma_start(outr[:, 0, :], res_sb[:, 0, :])
                # cu=1 row reuses the single pstrow bank
                for nk in range(8):
                    nc.tensor.transpose(
                        pstrow[:, nk * P:(nk + 1) * P],
                        o_sb[:, nk, P:2 * P], ident[:])
                nc.vector.tensor_tensor(
                    res_sb[:, 1, :], pstrow[:], x_sb[:, 1, :], ALU.add)
                nc.sync.dma_start(outr[:, 1, :], res_sb[:, 1, :])

            def out_state(img, vto8, pt8, x_sb):
                o_sb = op.tile([P, 8, C], BF16, tag="o")
                pstrow = pst_p.tile([P, N], BF16, tag="s")
                return (img, vto8, pt8, x_sb, o_sb, pstrow)

            # depth-3 interleaved pipeline: iteration k emits logits(k)
            # paced by the exp chain; qkv-A(k+1) at mk0, GN(k+2) at mk1,
            # out(k-1) chunks at mk2..7, qkv-B(k+1) after mk7.
            imgs = [i % BPC for i in range(BPC * repeat)]
            n_img = len(imgs)
            emit_consts()
            x_t = {0: emit_x(imgs[0])}
            warm = consts.tile([P, 1], F32)
            nc.vector.memset(warm[:], 0.0)
            nc.vector.memset(expb[:], EXPB)
            nc.scalar.activation(warm[:], warm[:], AF.Exp)
            h_t = {0: emit_gn_h(x_t[0], first=True)}

            def full_qkv(h_sb):
                qk8 = qkp.tile([P, 4, N], FP8, tag="qk")
                vto8 = vtp.tile([P, 8, C + 1], FP8, tag="vto")
                emit_qkv_a(h_sb, qk8, vto8)
                emit_qkv_b(h_sb, qk8)
                return qk8, vto8

            qk_t = {0: full_qkv(h_t.pop(0))}
            if n_img > 1:
                x_t[1] = emit_x(imgs[1])
                h_t[1] = emit_gn_h(x_t[1])
            if n_img > 2:
                x_t[2] = emit_x(imgs[2])
            prev = None  # out-state of image k-1
            for k in range(n_img):
                if k + 3 < n_img:
                    x_t[k + 3] = emit_x(imgs[k + 3])
                qk_n = qkp.tile([P, 4, N], FP8, tag="qk")
                vto_n = vtp.tile([P, 8, C + 1], FP8, tag="vto")
                closures = [None] * 9

                def c_qkv_a(k=k, qk_n=qk_n, vto_n=vto_n):
                    if k + 1 < n_img:
                        emit_qkv_a(h_t[k + 1], qk_n, vto_n)

                def c_gn(k=k):
                    if k + 2 < n_img:
                        h_t[k + 2] = emit_gn_h(x_t[k + 2])

                def c_qkv_b(k=k, qk_n=qk_n):
                    if k + 1 < n_img:
                        emit_qkv_b(h_t.pop(k + 1), qk_n)

                closures[0] = c_qkv_a
                closures[1] = c_gn
                if prev is not None:
                    closures[2] = lambda p=prev: out_chunk(p, (0, 1))
                    closures[3] = lambda p=prev: out_chunk(p, (2,))
                    closures[4] = lambda p=prev: out_chunk(p, (3, 4))
                    closures[5] = lambda p=prev: out_chunk(p, (5,))
                    closures[6] = lambda p=prev: out_chunk(p, (6, 7))

                    def c_b_tail(p=prev, k=k, qk_n=qk_n):
                        c_qkv_b(k, qk_n)
                        out_tail(p)
                    closures[8] = c_b_tail
                else:
                    closures[8] = c_qkv_b
                qk8, vto8 = qk_t.pop(k)
                pt8 = emit_logits(qk8, closures)
                if k + 1 < n_img:
                    qk_t[k + 1] = (qk_n, vto_n)
                prev = out_state(imgs[k], vto8, pt8, x_t.pop(k))
            out_chunk(prev, (0, 1))
            out_chunk(prev, (2, 3))
            out_chunk(prev, (4, 5))
            out_chunk(prev, (6, 7))
            out_tail(prev)

    nc.compile()
    return nc


def _build_runner(repeat=1):
    """Build nc once and wrap it in a persistent jitted 8-core SPMD callable."""
    nc = _build_program(repeat)
    bass2jax.install_neuronx_cc_hook()

    partition_name = (nc.partition_id_tensor.name
                      if nc.partition_id_tensor else None)
    in_names, out_names, out_avals = [], [], []
    for alloc in nc.m.functions[0].allocations:
        if not isinstance(alloc, mybir.MemoryLocationSet):
            continue
        name = alloc.memorylocations[0].name
        if alloc.kind == "ExternalInput":
            if name != partition_name:
                in_names.append(name)
        elif alloc.kind == "ExternalOutput":
            out_names.append(name)
            out_avals.append(jax.core.ShapedArray(
                tuple(alloc.tensor_shape), mybir.dt.np(alloc.dtype)))
    n_params = len(in_names)
    all_in_names = tuple(in_names) + tuple(out_names)
    if partition_name is not None:
        all_in_names = all_in_names + (partition_name,)

    def _body(*args):
        operands = list(args)
        if partition_name is not None:
            operands.append(bass2jax.partition_id_tensor())
        return tuple(bass2jax._bass_exec_p.bind(
            *operands,
            out_avals=tuple(out_avals),
            in_names=all_in_names,
            out_names=tuple(out_names),
            lowering_input_output_aliases=(),
            sim_require_finite=True,
            sim_require_nnan=True,
            nc=nc,
        ))

    devices = jax.devices()[:NCORES]
    mesh = Mesh(np.asarray(devices), ("core",))
    nin = n_params + len(out_names)
    sharded = jax.jit(
        shard_map(_body, mesh=mesh,
                  in_specs=(PartitionSpec("core"),) * nin,
                  out_specs=(PartitionSpec("core"),) * len(out_names),
                  check_rep=False),
        keep_unused=True,
    )
    from jax.sharding import NamedSharding
    shard = NamedSharding(mesh, PartitionSpec("core"))
    zeros_dev = [
        jax.device_put(
            np.zeros((NCORES * a.shape[0], *a.shape[1:]), a.dtype), shard)
        for a in out_avals
    ]
    return {"sharded": sharded, "in_names": in_names,
            "out_names": out_names, "out_avals": out_avals,
            "zeros_dev": zeros_dev, "mesh": mesh, "nc": nc}


def _get_runner(repeat=1):
    global _cached
    if _cached is None:
        _cached = {}
    if repeat not in _cached:
        _cached[repeat] = _build_runner(repeat)
    return _cached[repeat]


def _run(in_maps):
    r = _get_runner()
    sharded, in_names, out_names, out_avals, zeros_dev = (
        r["sharded"], r["in_names"], r["out_names"], r["out_avals"],
        r["zeros_dev"])
    concat_in = [
        np.concatenate([np.asarray(m[name]) for m in in_maps], axis=0)
        for name in in_names
    ]
    out_arrs = sharded(*concat_in, *zeros_dev)
    return {
        name: np.asarray(out_arrs[i]).reshape(
            NCORES, *out_avals[i].shape)
        for i, name in enumerate(out_names)
    }


def _pack_c(v):
    # [C] -> [P, NT] with c = t*128 + p
    return np.ascontiguousarray(v.reshape(NT, P).T)


def _pack_w(wT, dt):
    # [C, O] -> [P, NT, O] with c = t*128 + p
    o = wT.shape[1]
    return np.ascontiguousarray(
        wT.reshape(NT, P, o).transpose(1, 0, 2)).astype(dt)


def make_in_maps(x, norm_w, norm_b, qkv_w, proj_w, proj_b):
    bf16 = ml_dtypes.bfloat16
    x = np.asarray(x, dtype=np.float32)
    norm_w = np.asarray(norm_w, dtype=np.float32)
    norm_b = np.asarray(norm_b, dtype=np.float32)
    qkv_w = np.asarray(qkv_w, dtype=np.float32)
    proj_w = np.asarray(proj_w, dtype=np.float32)
    proj_b = np.asarray(proj_b, dtype=np.float32)

    wqk = _pack_w(qkv_w[:2 * C].T, bf16)     # [P, NT, 512]
    # fold proj into V: (Wp @ Wv).T, computed in float64 for exactness
    wvp = (proj_w.astype(np.float64) @ qkv_w[2 * C:].astype(np.float64))
    wv = _pack_w(wvp.astype(np.float32).T, bf16)  # [P, NT, 256]

    cidx = np.arange(C)
    sel = np.zeros((P, NT, G), np.float32)
    sel[cidx % P, cidx // P, cidx // GS] = 1.0 / GS
    selb = np.zeros((P, C), np.float32)
    selb[cidx // GS, cidx] = 1.0

    aff = np.concatenate(
        [_pack_c(norm_w), _pack_c(norm_b), _pack_c(proj_b)],
        axis=1).astype(np.float32)           # [P, 6]
    ident = np.eye(P, dtype=bf16)

    # fold proj_b into the shipped input: xpb = x + proj_b (bf16)
    xpb = (x + proj_b[None, :, None, None]).astype(bf16).reshape(B, C, N)
    shared = {"wqk": wqk, "wv": wv, "sel": sel, "selb": selb,
              "aff": aff, "ident": ident}
    return [
        {"xpb": np.ascontiguousarray(xpb[c * BPC:(c + 1) * BPC]), **shared}
        for c in range(NCORES)
    ]


def kernel(x, norm_w, norm_b, qkv_w, proj_w, proj_b):
    in_maps = make_in_maps(x, norm_w, norm_b, qkv_w, proj_w, proj_b)
    outs = _run(in_maps)
    return outs["out"].reshape(B, C, 32, 32).astype(np.float32)


# revision 5
# speedup vs baseline: 6.8323x; 6.8323x over previous
"""AttentionBlock (GroupNorm -> 1x1 QKV -> softmax attention -> proj -> residual)
on Trainium2, data-parallel over batch: 32 images across 8 NeuronCores (4 per core).

v2: fp8e4m3 DoubleRow matmuls (K=256 per matmul, 0.5 cyc/col) for the
logits and attn@V stages; bf16 for h/Wqk/Wv generation matmuls; bf16 input
(x + proj_b folded on host) and bf16 output; exp with -1.5 bias shift
(cancels in softmax) so fp8 exp outputs stay in e4m3 range.  All PSUM
drains on DVE/ACT (GPSIMD cannot access PSUM on TRN2).

Self-contained: hardcodes B=32, C=256, H=W=32, GROUPS=8, EPS=1e-5.
"""

import numpy as np
import ml_dtypes
import jax
from jax.experimental.shard_map import shard_map
from jax.sharding import Mesh, PartitionSpec

import concourse.bass as bass
import concourse.tile as tile
from concourse import bacc, mybir
from concourse import bass2jax

F32 = mybir.dt.float32
BF16 = mybir.dt.bfloat16
FP8 = mybir.dt.float8e4
AF = mybir.ActivationFunctionType
ALU = mybir.AluOpType
PM = mybir.MatmulPerfMode

NCORES = 8
B = 32
BPC = B // NCORES  # images per core
C = 256
N = 1024           # H*W
G = 8              # groups
GS = C // G        # 32 channels per group
EPS = 1e-5
P = 128
NT = C // P        # 2 channel tiles
SCALE = C ** -0.5  # 1/16
EXPB = -2.5        # exp bias shift; cancels in softmax, keeps fp8 PT well in range

_cached = None


def _build_program(repeat=1):
    nc = bacc.Bacc("TRN2", target_bir_lowering=False, debug=False,
                   num_devices=NCORES)

    # xpb = x + proj_b (folded on host), bf16
    xpb_d = nc.dram_tensor("xpb", [BPC, C, N], BF16, kind="ExternalInput")
    wqk_d = nc.dram_tensor("wqk", [P, NT, 2 * C], BF16, kind="ExternalInput")
    # wv holds (proj_w @ v_w).T -- proj folded into V (attention only mixes
    # spatially: Wp @ (attn @ (Wv h)) = attn @ ((Wp Wv) h))
    wv_d = nc.dram_tensor("wv", [P, NT, C], BF16, kind="ExternalInput")
    sel_d = nc.dram_tensor("sel", [P, NT, G], F32, kind="ExternalInput")
    selb_d = nc.dram_tensor("selb", [P, C], F32, kind="ExternalInput")
    # aff cols: [norm_w (NT), norm_b (NT), proj_b (NT)]
    aff_d = nc.dram_tensor("aff", [P, 3 * NT], F32, kind="ExternalInput")
    ident_d = nc.dram_tensor("ident", [P, P], BF16, kind="ExternalInput")
    out_d = nc.dram_tensor("out", [BPC, C, N], BF16, kind="ExternalOutput")

    with tile.TileContext(nc) as tc:
        with (
            tc.tile_pool(name="consts", bufs=1) as consts,
            tc.tile_pool(name="xp", bufs=5) as xp,
            tc.tile_pool(name="gn", bufs=3) as gn,
            tc.tile_pool(name="hp", bufs=3) as hp,
            tc.tile_pool(name="qkp", bufs=3) as qkp,
            tc.tile_pool(name="vtp", bufs=3) as vtp,
            tc.tile_pool(name="ptp", bufs=3) as ptp,
            tc.tile_pool(name="op", bufs=3) as op,
            tc.tile_pool(name="resp", bufs=3) as resp,
            tc.tile_pool(name="recp", bufs=4) as recp,
            tc.tile_pool(name="psb", bufs=2, space="PSUM") as psb,
            tc.tile_pool(name="psv_p", bufs=1, space="PSUM") as psv_p,
            tc.tile_pool(name="pso_p", bufs=2, space="PSUM") as pso_p,
            tc.tile_pool(name="pst_p", bufs=1, space="PSUM") as pst_p,
        ):
            wqk = consts.tile([P, NT, 2 * C], BF16)
            wv = consts.tile([P, NT, C], BF16)
            sel = consts.tile([P, NT, G], F32)
            selb = consts.tile([P, C], F32)
            aff = consts.tile([P, 3 * NT], F32)
            ident = consts.tile([P, P], BF16)
            expb = consts.tile([P, 1], F32)

            def emit_consts():
                nc.sync.dma_start(sel[:], sel_d.ap())
                nc.sync.dma_start(selb[:], selb_d.ap())
                nc.sync.dma_start(aff[:], aff_d.ap())
                nc.gpsimd.dma_start(wqk[:], wqk_d.ap())
                nc.gpsimd.dma_start(wv[:], wv_d.ap())
                nc.gpsimd.dma_start(ident[:], ident_d.ap())

            def emit_x(img):
                x_sb = xp.tile([P, NT, N], BF16, tag="x")
                xr = xpb_d.ap()[img].rearrange("(t p) n -> p t n", p=P)
                nc.sync.dma_start(x_sb[:], xr[:, :, :])
                return x_sb

            def emit_gn_h(x_sb, first=False):
                """GroupNorm stats on xpb -> per-channel affine -> h (bf16).

                xpb = x + pb, so mean' = mean(x) + pb per channel; the h
                affine needs B = b - (mu_g + pb)*A and the group stats need
                per-channel E[x^2] = var' + (mean' - pb)^2.
                """
                bst = gn.tile([P, NT, 2, 6], F32, tag="bst")
                for t in range(NT):
                    for s in range(2):
                        nc.vector.bn_stats(
                            bst[:, t, s, :], x_sb[:, t, s * 512:(s + 1) * 512])
                cmv = gn.tile([P, NT, 2], F32, tag="cmv")
                for t in range(NT):
                    nc.vector.bn_aggr(cmv[:, t, :], bst[:, t, :, :])
                # ex2 columns: [meanX_c, E[x^2]_c] with meanX = mean' - pb
                ex2 = gn.tile([P, NT, 2], F32, tag="ex2")
                for t in range(NT):
                    nc.vector.tensor_tensor(
                        ex2[:, t, 0:1], cmv[:, t, 0:1],
                        aff[:, 2 * NT + t:2 * NT + t + 1], ALU.subtract)
                    nc.vector.tensor_mul(
                        ex2[:, t, 1:2], ex2[:, t, 0:1], ex2[:, t, 0:1])
                    nc.vector.tensor_add(
                        ex2[:, t, 1:2], ex2[:, t, 1:2], cmv[:, t, 1:2])
                # group stats = (1/GS) * sel.T @ ex2 -> psum [G, 2]
                psg = psv_p.tile([G, 2], F32, tag="g")
                for t in range(NT):
                    nc.tensor.matmul(psg[:], sel[:, t, :], ex2[:, t, :],
                                     start=(t == 0), stop=(t == NT - 1))
                # gsb cols: [mean_g, rstd_g, v, tmp]; rows 8..127 zero (pad
                # for matmul).  rstd via DVE-only Newton rsqrt so Exp stays
                # the single ACT table set.
                gsb = gn.tile([P, 4], F32, tag="gsb")
                nc.vector.memset(gsb[:], 0.0)
                nc.vector.tensor_copy(gsb[0:G, 0:1], psg[:, 0:1])
                nc.vector.tensor_mul(
                    gsb[0:G, 3:4], gsb[0:G, 0:1], gsb[0:G, 0:1])
                nc.vector.tensor_tensor(
                    gsb[0:G, 2:3], psg[:, 1:2], gsb[0:G, 3:4], ALU.subtract)
                nc.vector.tensor_scalar_add(gsb[0:G, 2:3], gsb[0:G, 2:3], EPS)
                nc.vector.reciprocal(gsb[0:G, 3:4], gsb[0:G, 2:3])
                nc.vector.tensor_scalar(
                    gsb[0:G, 1:2], gsb[0:G, 3:4], 1.0, 0.5, ALU.add, ALU.mult)
                for _ in range(2):
                    nc.vector.tensor_mul(
                        gsb[0:G, 3:4], gsb[0:G, 1:2], gsb[0:G, 1:2])
                    nc.vector.tensor_mul(
                        gsb[0:G, 3:4], gsb[0:G, 3:4], gsb[0:G, 2:3])
                    nc.vector.tensor_scalar(
                        gsb[0:G, 3:4], gsb[0:G, 3:4], -0.5, 1.5,
                        ALU.mult, ALU.add)
                    nc.vector.tensor_mul(
                        gsb[0:G, 1:2], gsb[0:G, 1:2], gsb[0:G, 3:4])
                # broadcast group -> channel: selb.T @ gsb -> [c, (mean,rstd)]
                AB = gn.tile([P, NT, 2], F32, tag="AB")
                h_sb = hp.tile([P, NT, N], BF16, tag="h")
                for cu in range(NT):
                    psc = psv_p.tile([P, 2], F32, tag="g")
                    nc.tensor.matmul(psc[:], selb[:, cu * P:(cu + 1) * P],
                                     gsb[:, 0:2], start=True, stop=True)
                    # A = rstd*w ; B = b - (mu_g + pb)*A
                    nc.vector.tensor_mul(
                        AB[:, cu, 0:1], psc[:, 1:2], aff[:, cu:cu + 1])
                    nc.vector.tensor_add(
                        AB[:, cu, 1:2], psc[:, 0:1],
                        aff[:, 2 * NT + cu:2 * NT + cu + 1])
                    nc.vector.tensor_mul(
                        AB[:, cu, 1:2], AB[:, cu, 1:2], AB[:, cu, 0:1])
                    nc.vector.tensor_tensor(
                        AB[:, cu, 1:2], aff[:, NT + cu:NT + cu + 1],
                        AB[:, cu, 1:2], ALU.subtract)
                    # h = A*xpb + B   (bf16, all-SBUF: fast DVE path)
                    nc.vector.tensor_scalar(
                        h_sb[:, cu, :], x_sb[:, cu, :],
                        AB[:, cu, 0:1], AB[:, cu, 1:2], ALU.mult, ALU.add)
                return h_sb

            def _drain(eng, dst, src):
                # psum -> sbuf drain on ACT (scalar.copy) or DVE (tensor_copy)
                if eng is nc.scalar:
                    eng.copy(dst, src)
                else:
                    eng.tensor_copy(dst, src)

            def emit_qkv_a(h_sb, qk8, vto8):
                """k as [P,512] halves via the pso pool (bf16 mms, fp8
                copies split ACT/DVE); vpT via psv pool."""
                def qk_group(ou, engs):
                    for half in range(2):
                        psq = pso_p.tile([P, 512], F32, tag="s")
                        for t in range(NT):
                            nc.tensor.matmul(
                                psq[:],
                                wqk[:, t, ou * P:(ou + 1) * P],
                                h_sb[:, t, half * 512:(half + 1) * 512],
                                start=(t == 0), stop=(t == NT - 1))
                        _drain(engs[half],
                               qk8[:, ou, half * 512:(half + 1) * 512],
                               psq[:])

                def vt_group(j, eng):
                    psv = psv_p.tile([P, 2, C], F32, tag="g")
                    for half in range(2):
                        nk = 2 * j + half
                        for t in range(NT):
                            nc.tensor.matmul(
                                psv[:, half, :],
                                h_sb[:, t, nk * P:(nk + 1) * P],
                                wv[:, t, :],
                                start=(t == 0), stop=(t == NT - 1))
                    _drain(eng, vto8[:, 2 * j:2 * j + 2, 0:C], psv[:])
                    nc.gpsimd.memset(
                        vto8[:, 2 * j:2 * j + 2, C:C + 1], 1.0)

                qk_group(2, (nc.scalar, nc.vector))
                qk_group(3, (nc.scalar, nc.vector))
                vt_group(0, nc.scalar)
                vt_group(1, nc.vector)
                vt_group(2, nc.scalar)
                vt_group(3, nc.vector)

            def emit_qkv_b(h_sb, qk8):
                """q (ou 0,1) as [P,512] halves via the pso pool."""
                for ou in (0, 1):
                    for half, eng in ((0, nc.scalar), (1, nc.vector)):
                        psq = pso_p.tile([P, 512], F32, tag="s")
                        for t in range(NT):
                            nc.tensor.matmul(
                                psq[:],
                                wqk[:, t, ou * P:(ou + 1) * P],
                                h_sb[:, t, half * 512:(half + 1) * 512],
                                start=(t == 0), stop=(t == NT - 1))
                        _drain(eng,
                               qk8[:, ou, half * 512:(half + 1) * 512],
                               psq[:])

            def emit_logits(qk8, closures):
                """logitsT [m, n] = k.T @ q (fp8 DoubleRow);
                PT = exp(logitsT/16 - 1.5) in fp8.  closures[mk] emit the
                neighbouring images' work between mk groups."""
                pt8 = ptp.tile([P, 8, N], FP8, tag="pt")
                for mk in range(8):
                    psl = psb.tile([P, N], F32, tag="b")
                    for s in range(4):
                        nc.tensor.matmul(
                            psl[:, s * 256:(s + 1) * 256],
                            qk8[:, 2:4, mk * P:(mk + 1) * P],
                            qk8[:, 0:2, s * 256:(s + 1) * 256],
                            start=True, stop=True, perf_mode=PM.DoubleRow)
                    nc.scalar.activation(pt8[:, mk, :], psl[:], AF.Exp,
                                         scale=SCALE, bias=expb[:])
                    if closures[mk] is not None:
                        closures[mk]()
                if closures[8] is not None:
                    closures[8]()
                return pt8

            def out_chunk(state, nks):
                """AV DoubleRow matmuls (vto carries a ones column for the
                softmax denominator) + rec + normalize, cu=0 transposes
                lagging one nk."""
                img, vto8, pt8, x_sb, o_sb, pstrow = state
                for nk in nks:
                    pso = pso_p.tile([P, 512], F32, tag="s")
                    for j in range(4):
                        nc.tensor.matmul(
                            pso[:, 0:C + 1],
                            pt8[:, 2 * j:2 * j + 2, nk * P:(nk + 1) * P],
                            vto8[:, 2 * j:2 * j + 2, :],
                            start=(j == 0), stop=(j == 3),
                            perf_mode=PM.DoubleRow)
                    rec = recp.tile([P, 1], F32, tag="rec")
                    nc.vector.reciprocal(rec[:], pso[:, C:C + 1])
                    nc.vector.tensor_scalar_mul(
                        o_sb[:, nk, :], pso[:, 0:C], rec[:])
                    if nk > 0:
                        nc.tensor.transpose(
                            pstrow[:, (nk - 1) * P:nk * P],
                            o_sb[:, nk - 1, 0:P], ident[:])

            def out_tail(state):
                img, vto8, pt8, x_sb, o_sb, pstrow = state
                res_sb = resp.tile([P, NT, N], BF16, tag="res")
                outr = out_d.ap()[img].rearrange("(t p) n -> p t n", p=P)
                nc.tensor.transpose(
                    pstrow[:, 7 * P:8 * P], o_sb[:, 7, 0:P], ident[:])
                nc.vector.tensor_tensor(
                    res_sb[:, 0, :], pstrow[:], x_sb[:, 0, :], ALU.add)
                nc.sync.dma_start(outr[:, 0, :], res_sb[:, 0, :])
                # cu=1 row reuses the single pstrow bank
                for nk in range(8):
                    nc.tensor.transpose(
                        pstrow[:, nk * P:(nk + 1) * P],
                        o_sb[:, nk, P:2 * P], ident[:])
                nc.vector.tensor_tensor(
                    res_sb[:, 1, :], pstrow[:], x_sb[:, 1, :], ALU.add)
                nc.sync.dma_start(outr[:, 1, :], res_sb[:, 1, :])

            def out_state(img, vto8, pt8, x_sb):
                o_sb = op.tile([P, 8, C], BF16, tag="o")
                pstrow = pst_p.tile([P, N], BF16, tag="s")
                return (img, vto8, pt8, x_sb, o_sb, pstrow)

            # depth-3 interleaved pipeline: iteration k emits logits(k)
            # paced by the exp chain; qkv-A(k+1) at mk0, GN(k+2) at mk1,
            # out(k-1) chunks at mk2..7, qkv-B(k+1) after mk7.
            imgs = [i % BPC for i in range(BPC * repeat)]
            n_img = len(imgs)
            emit_consts()
            x_t = {0: emit_x(imgs[0])}
            warm = consts.tile([P, 1], F32)
            nc.vector.memset(warm[:], 0.0)
            nc.vector.memset(expb[:], EXPB)
            nc.scalar.activation(warm[:], warm[:], AF.Exp)
            h_t = {0: emit_gn_h(x_t[0], first=True)}

            def full_qkv(h_sb):
                qk8 = qkp.tile([P, 4, N], FP8, tag="qk")
                vto8 = vtp.tile([P, 8, C + 1], FP8, tag="vto")
                emit_qkv_a(h_sb, qk8, vto8)
                emit_qkv_b(h_sb, qk8)
                return qk8, vto8

            qk_t = {0: full_qkv(h_t.pop(0))}
            if n_img > 1:
                x_t[1] = emit_x(imgs[1])
                h_t[1] = emit_gn_h(x_t[1])
            if n_img > 2:
                x_t[2] = emit_x(imgs[2])
            prev = None  # out-state of image k-1
            for k in range(n_img):
                if k + 3 < n_img:
                    x_t[k + 3] = emit_x(imgs[k + 3])
                qk_n = qkp.tile([P, 4, N], FP8, tag="qk")
                vto_n = vtp.tile([P, 8, C + 1], FP8, tag="vto")
                closures = [None] * 9

                def c_qkv_a(k=k, qk_n=qk_n, vto_n=vto_n):
                    if k + 1 < n_img:
                        emit_qkv_a(h_t[k + 1], qk_n, vto_n)

                def c_gn(k=k):
                    if k + 2 < n_img:
                        h_t[k + 2] = emit_gn_h(x_t[k + 2])

                def c_qkv_b(k=k, qk_n=qk_n):
                    if k + 1 < n_img:
                        emit_qkv_b(h_t.pop(k + 1), qk_n)

                closures[0] = c_qkv_a
                closures[1] = c_gn
                if prev is not None:
                    closures[2] = lambda p=prev: out_chunk(p, (0, 1))
                    closures[3] = lambda p=prev: out_chunk(p, (2,))
                    closures[4] = lambda p=prev: out_chunk(p, (3, 4))
                    closures[5] = lambda p=prev: out_chunk(p, (5,))
                    closures[6] = lambda p=prev: out_chunk(p, (6, 7))

                    def c_b_tail(p=prev, k=k, qk_n=qk_n):
                        c_qkv_b(k, qk_n)
                        out_tail(p)
                    closures[8] = c_b_tail
                else:
                    closures[8] = c_qkv_b
                qk8, vto8 = qk_t.pop(k)
                pt8 = emit_logits(qk8, closures)
                if k + 1 < n_img:
                    qk_t[k + 1] = (qk_n, vto_n)
                prev = out_state(imgs[k], vto8, pt8, x_t.pop(k))
            out_chunk(prev, (0, 1))
            out_chunk(prev, (2, 3))
            out_chunk(prev, (4, 5))
            out_chunk(prev, (6, 7))
            out_tail(prev)

    nc.compile()
    return nc


def _build_runner(repeat=1):
    """Build nc once and wrap it in a persistent jitted 8-core SPMD callable."""
    nc = _build_program(repeat)
    bass2jax.install_neuronx_cc_hook()

    partition_name = (nc.partition_id_tensor.name
                      if nc.partition_id_tensor else None)
    in_names, out_names, out_avals = [], [], []
    for alloc in nc.m.functions[0].allocations:
        if not isinstance(alloc, mybir.MemoryLocationSet):
            continue
        name = alloc.memorylocations[0].name
        if alloc.kind == "ExternalInput":
            if name != partition_name:
                in_names.append(name)
        elif alloc.kind == "ExternalOutput":
            out_names.append(name)
            out_avals.append(jax.core.ShapedArray(
                tuple(alloc.tensor_shape), mybir.dt.np(alloc.dtype)))
    n_params = len(in_names)
    all_in_names = tuple(in_names) + tuple(out_names)
    if partition_name is not None:
        all_in_names = all_in_names + (partition_name,)

    def _body(*args):
        operands = list(args)
        if partition_name is not None:
            operands.append(bass2jax.partition_id_tensor())
        return tuple(bass2jax._bass_exec_p.bind(
            *operands,
            out_avals=tuple(out_avals),
            in_names=all_in_names,
            out_names=tuple(out_names),
            lowering_input_output_aliases=(),
            sim_require_finite=True,
            sim_require_nnan=True,
            nc=nc,
        ))

    devices = jax.devices()[:NCORES]
    mesh = Mesh(np.asarray(devices), ("core",))
    nin = n_params + len(out_names)
    sharded = jax.jit(
        shard_map(_body, mesh=mesh,
                  in_specs=(PartitionSpec("core"),) * nin,
                  out_specs=(PartitionSpec("core"),) * len(out_names),
                  check_rep=False),
        keep_unused=True,
    )
    from jax.sharding import NamedSharding
    shard = NamedSharding(mesh, PartitionSpec("core"))
    zeros_dev = [
        jax.device_put(
            np.zeros((NCORES * a.shape[0], *a.shape[1:]), a.dtype), shard)
        for a in out_avals
    ]
    return {"sharded": sharded, "in_names": in_names,
            "out_names": out_names, "out_avals": out_avals,
            "zeros_dev": zeros_dev, "mesh": mesh, "nc": nc}


def _get_runner(repeat=1):
    global _cached
    if _cached is None:
        _cached = {}
    if repeat not in _cached:
        _cached[repeat] = _build_runner(repeat)
    return _cached[repeat]


def _run(in_maps):
    r = _get_runner()
    sharded, in_names, out_names, out_avals, zeros_dev = (
        r["sharded"], r["in_names"], r["out_names"], r["out_avals"],
        r["zeros_dev"])
    concat_in = [
        np.concatenate([np.asarray(m[name]) for m in in_maps], axis=0)
        for name in in_names
    ]
    out_arrs = sharded(*concat_in, *zeros_dev)
    return {
        name: np.asarray(out_arrs[i]).reshape(
            NCORES, *out_avals[i].shape)
        for i, name in enumerate(out_names)
    }


def _pack_c(v):
    # [C] -> [P, NT] with c = t*128 + p
    return np.ascontiguousarray(v.reshape(NT, P).T)


def _pack_w(wT, dt):
    # [C, O] -> [P, NT, O] with c = t*128 + p
    o = wT.shape[1]
    return np.ascontiguousarray(
        wT.reshape(NT, P, o).transpose(1, 0, 2)).astype(dt)


def make_in_maps(x, norm_w, norm_b, qkv_w, proj_w, proj_b):
    bf16 = ml_dtypes.bfloat16
    x = np.asarray(x, dtype=np.float32)
    norm_w = np.asarray(norm_w, dtype=np.float32)
    norm_b = np.asarray(norm_b, dtype=np.float32)
    qkv_w = np.asarray(qkv_w, dtype=np.float32)
    proj_w = np.asarray(proj_w, dtype=np.float32)
    proj_b = np.asarray(proj_b, dtype=np.float32)

    wqk = _pack_w(qkv_w[:2 * C].T, bf16)     # [P, NT, 512]
    # fold proj into V: (Wp @ Wv).T, computed in float64 for exactness
    wvp = (proj_w.astype(np.float64) @ qkv_w[2 * C:].astype(np.float64))
    wv = _pack_w(wvp.astype(np.float32).T, bf16)  # [P, NT, 256]

    cidx = np.arange(C)
    sel = np.zeros((P, NT, G), np.float32)
    sel[cidx % P, cidx // P, cidx // GS] = 1.0 / GS
    selb = np.zeros((P, C), np.float32)
    selb[cidx // GS, cidx] = 1.0

    aff = np.concatenate(
        [_pack_c(norm_w), _pack_c(norm_b), _pack_c(proj_b)],
        axis=1).astype(np.float32)           # [P, 6]
    ident = np.eye(P, dtype=bf16)

    # fold proj_b into the shipped input: xpb = x + proj_b (bf16)
    xpb = (x + proj_b[None, :, None, None]).astype(bf16).reshape(B, C, N)
    shared = {"wqk": wqk, "wv": wv, "sel": sel, "selb": selb,
              "aff": aff, "ident": ident}
    return [
        {"xpb": np.ascontiguousarray(xpb[c * BPC:(c + 1) * BPC]), **shared}
        for c in range(NCORES)
    ]


def kernel(x, norm_w, norm_b, qkv_w, proj_w, proj_b):
    in_maps = make_in_maps(x, norm_w, norm_b, qkv_w, proj_w, proj_b)
    outs = _run(in_maps)
    return outs["out"].reshape(B, C, 32, 32).astype(np.float32)
